# revision 1
# baseline (speedup 1.0000x reference)
"""Trainium2 Bass kernel for nn_AttentionBlock (B=4, C=512, N=2048, H=8, DK=64).

Computation (see reference):
  xt = x.transpose(0,2,1)            # [B, N, C]
  qkv = xt @ Wp.T + bp               # [B, N, 3*H*DK], split per head into q,k,v
  S[b,i,j,h] = q[b,i,h,:]. k[b,j,h,:] * DK**-0.5
  P = softmax over i (the QUERY axis)
  O[b,i,h,:] = sum_j P[b,i,j,h] v[b,j,h,:]
  out = (O.reshape(b,n,H*DK) @ Wo.T + bo + xt).transpose(0,2,1)

Sharding: 8 cores = (batch b = core//2) x (head-group g = core%2, 4 heads each).
Each core writes two f16 partial resT[c_out, n] outputs (one per head pair);
host sums the four partials per batch and adds bias + residual.

On-device layouts (per core):
  x[b] is [C, N] already = [contraction, tokens] for the QKV projection.
  qk_sb[:, t, :]  - [128, 2048] f16, t in {qq0, kk0, qq1, kk1}; each tile
                    holds two heads' q^T (or k^T) stacked on partitions
                    (head A rows 0-63, head B rows 64-127) - enables
                    row-packed (tile_position) K=64 S matmuls.
  v_sb[:, jt, :]  - [128 tokens, 16, 256] f16, v in [token, head*64+d] layout,
                    produced by using x-tiles as the stationary operand.
  S_T[j, i] in PSUM per (pair, jt): softmax over i == free axis. exp on ACT
  (scale=1/8 folded in, accum_out gives row sums). Normalization folded into
  v: vp = v * (128/sumE); the 1/128 is applied at O evacuation.
  O_T accumulates col-packed: head A -> psum partitions 0-63, head B 64-127.

Scheduling notes (engines are in-order; ACT exp is the bottleneck at
~128 exp ops of [128,1024]):
  - PV(jt) is emitted AFTER S(jt+1) so the PE never stalls waiting exp(jt).
  - Projection / output-projection work that rides under the attention loops
    goes through the "o" PSUM slot chain (before the O accumulator of that
    pair is allocated), never through the "s" slots that feed exp.
  - Pair 1's output projection runs in the tail where ACT is idle, so its
    PSUM evacuations alternate between ACT and DVE.
"""

import os
import numpy as np

import concourse.bass as bass
import concourse.tile as tile
from concourse import bacc, mybir
from concourse.bass_utils import run_bass_kernel_spmd

F32 = mybir.dt.float32
F16 = mybir.dt.float16
AF = mybir.ActivationFunctionType
ALU = mybir.AluOpType

B, C, N = 4, 512, 2048
H, DK = 8, 64
N_CORES = 8
SCALE = DK ** -0.5  # 0.125
VP_SCALE = 128.0    # keeps vp = v/sumE out of fp16 subnormal range

# module-level stash so test.py can read profiling info
LAST_RESULT = None
_NC = None


def _build_nc():
    nc = bacc.Bacc("TRN2", target_bir_lowering=False, debug=False,
                   num_devices=N_CORES)

    x16 = nc.dram_tensor("x16", [C, N], F16, kind="ExternalInput").ap()
    wqk = nc.dram_tensor("wqk", [C, 512], F16, kind="ExternalInput").ap()
    bqk = nc.dram_tensor("bqk", [128, 4], F32, kind="ExternalInput").ap()
    wv = nc.dram_tensor("wv", [C, 256], F16, kind="ExternalInput").ap()
    bpv = nc.dram_tensor("bpv", [1, 256], F16, kind="ExternalInput").ap()
    ones = nc.dram_tensor("ones", [1, 128], F16, kind="ExternalInput").ap()
    zrow = nc.dram_tensor("zrow", [1, 512], F16, kind="ExternalInput").ap()
    wo = nc.dram_tensor("wo", [256, C], F16, kind="ExternalInput").ap()
    out_a = nc.dram_tensor("out_a", [C, N], F16, kind="ExternalOutput").ap()
    out_b = nc.dram_tensor("out_b", [C, N], F16, kind="ExternalOutput").ap()

    with tile.TileContext(nc) as tc:
        with (
            tc.tile_pool(name="consts", bufs=1) as consts,
            tc.tile_pool(name="qkpool", bufs=1) as qkpool,
            tc.tile_pool(name="vpool", bufs=1) as vpool,
            tc.tile_pool(name="epool", bufs=20) as epool,
            tc.tile_pool(name="vppool", bufs=3) as vppool,
            tc.tile_pool(name="outpool", bufs=6) as outpool,
            tc.tile_pool(name="smalls", bufs=24) as smalls,
            tc.tile_pool(name="psum", bufs=1, space="PSUM") as pp,
        ):
            # ---- loads: tiny constants first (the bias feeds the very
            # first PSUM evacuation), weights per c-tile, x in column halves,
            # so the first projection blocks start after ~0.5 MB of DMA ----
            ones_sb = consts.tile([1, 128], F16)
            nc.sync.dma_start(ones_sb[:], ones[:])
            bqk_sb = consts.tile([128, 4], F32)
            nc.sync.dma_start(bqk_sb[:], bqk[:])
            bpv_sb = consts.tile([1, 256], F16)
            nc.sync.dma_start(bpv_sb[:], bpv[:])
            zrow_sb = consts.tile([1, 512], F16)
            nc.sync.dma_start(zrow_sb[:], zrow[:])
            wqk_sb = consts.tile([128, 4, 512], F16)
            x_sb = consts.tile([128, 4, N], F16)
            for ct in range(4):
                nc.sync.dma_start(wqk_sb[:, ct],
                                  wqk[ct * 128:(ct + 1) * 128, :])
                nc.sync.dma_start(
                    x_sb[:, ct, 0:1024],
                    x16[ct * 128:(ct + 1) * 128, 0:1024])
            for ct in range(4):
                nc.sync.dma_start(
                    x_sb[:, ct, 1024:2048],
                    x16[ct * 128:(ct + 1) * 128, 1024:2048])
            wv_sb = consts.tile([128, 4, 256], F16)
            nc.sync.dma_start(wv_sb[:], wv.rearrange("(co ci) f -> ci co f", ci=128))
            wo_sb = consts.tile([128, 2, C], F16)
            nc.sync.dma_start(wo_sb[:], wo.rearrange("(ko ki) m -> ki ko m", ki=128))

            qk_sb = qkpool.tile([128, 4, N], F16)
            v_sb = vpool.tile([128, 16, 256], F16)
            o_sb = qkpool.tile([128, 2, N], F16, name="o_sb")

            # warm the ACT exp table while DMAs run
            warm = smalls.tile([1, 128], F16, tag="warm", name="warm")
            nc.scalar.activation(warm[:], ones_sb[:], AF.Exp)

            def qk_proj(ft, blk, tag, evac="vector"):
                # qk_sb[:, ft, blk*1024:+1024] = (wqk tile).T @ x + bias
                if tag == "s":
                    ps = pp.tile([128, 1024], F32, tag="s", bufs=2, name="ps_qk")
                else:
                    ps = pp.tile([128, N], F32, tag="o", bufs=1,
                                 name="ps_qk_o")[:, :1024]
                for q in range(2):
                    nch = 2 * blk + q
                    for ct in range(4):
                        nc.tensor.matmul(
                            ps[:, q * 512:(q + 1) * 512],
                            lhsT=wqk_sb[:, ct, ft * 128:(ft + 1) * 128],
                            rhs=x_sb[:, ct, nch * 512:(nch + 1) * 512],
                            start=(ct == 0), stop=(ct == 3),
                        )
                if evac == "scalar":
                    # scalar engine is idle in the prologue; parallelizes
                    # the two critical-path evacuations
                    nc.scalar.add(
                        qk_sb[:, ft, blk * 1024:(blk + 1) * 1024],
                        ps[:], bqk_sb[:, ft:ft + 1])
                else:
                    nc.vector.tensor_scalar(
                        qk_sb[:, ft, blk * 1024:(blk + 1) * 1024],
                        ps[:], bqk_sb[:, ft:ft + 1], None, ALU.add,
                    )

            def v_proj(nt, tag):
                # v_sb[:, nt] = x_tile.T @ wv + bpv -> [128 tokens, 256]
                if tag == "s":
                    ps = pp.tile([128, 1024], F32, tag="s", bufs=2, name="ps_v")
                else:
                    ps = pp.tile([128, N], F32, tag="o", bufs=1,
                                 name="ps_v_o")[:, :1024]
                for ct in range(4):
                    nc.tensor.matmul(
                        ps[:, :256],
                        lhsT=x_sb[:, ct, nt * 128:(nt + 1) * 128],
                        rhs=wv_sb[:, ct, :],
                        start=(ct == 0), stop=False,
                    )
                nc.tensor.matmul(
                    ps[:, :256], lhsT=ones_sb[:1, :], rhs=bpv_sb[:1, :],
                    start=False, stop=True,
                )
                nc.vector.tensor_copy(v_sb[:, nt, :], ps[:, :256])

            def out_proj_unit(p_, cot, ic, tag="s", engine="vector"):
                # one (cot, ic) block of this pair's partial resT -> DRAM f16
                dst = out_a if p_ == 0 else out_b
                if tag == "s":
                    ps = pp.tile([128, 1024], F32, tag="s", bufs=2,
                                 name="ps_out")[:, :512]
                else:
                    ps = pp.tile([128, N], F32, tag="o", bufs=1,
                                 name="ps_out_o")[:, :512]
                nc.tensor.matmul(
                    ps[:],
                    lhsT=wo_sb[:, p_, cot * 128:(cot + 1) * 128],
                    rhs=o_sb[:, p_, ic * 512:(ic + 1) * 512],
                    start=True, stop=True,
                )
                out_t = outpool.tile([128, 512], F16, tag="outsb", name="out_t")
                if engine == "scalar":
                    nc.scalar.copy(out_t[:], ps[:])
                else:
                    nc.vector.tensor_copy(out_t[:], ps[:])
                nc.sync.dma_start(
                    dst[cot * 128:(cot + 1) * 128, ic * 512:(ic + 1) * 512],
                    out_t[:])

            def attention_pair(p_, prework=(), extra_work=None,
                               jt0_hook=None):
                # prework: callables using the "o" psum slot, emitted
                # interleaved with the first s_exp steps BEFORE the O
                # accumulator is allocated. extra_work: {jt: [callables]}.
                qq = qk_sb[:, 2 * p_]
                kk = qk_sb[:, 2 * p_ + 1]
                es = {}
                emitted = [0]

                def s_exp(jt, hook=None):
                    # ih-major so a hook can run after the first i-half's
                    # exps are emitted (used to slot the b1 qk-projection
                    # blocks under jt0's first exps).
                    e_ts = [epool.tile([128, N], F16, tag="e", name="e_t")
                            for _ in range(2)]
                    accs = [[], []]
                    recs = []
                    for ih in range(2):
                        for h in range(2):
                            rp = 64 * h
                            s_ps = pp.tile([128, 1024], F32, tag="s", bufs=2,
                                           name="s_ps")
                            for q in range(2):
                                ic = 2 * ih + q
                                nc.tensor.matmul(
                                    s_ps[:, q * 512:(q + 1) * 512],
                                    lhsT=kk[rp:rp + 64, jt * 128:(jt + 1) * 128],
                                    rhs=qq[rp:rp + 64, ic * 512:(ic + 1) * 512],
                                    start=True, stop=True,
                                    tile_position=(rp, 0),
                                )
                            acc = smalls.tile([128, 1], F32, tag="acc",
                                              bufs=8, name="acc")
                            nc.scalar.activation(
                                e_ts[h][:, ih * 1024:(ih + 1) * 1024], s_ps[:],
                                AF.Exp, scale=SCALE, accum_out=acc,
                            )
                            accs[h].append(acc)
                        if ih == 0 and hook is not None:
                            hook()
                    for h in range(2):
                        ssum = smalls.tile([128, 1], F32, tag="ssum", bufs=8,
                                           name="ssum")
                        nc.vector.tensor_add(ssum[:], accs[h][0][:],
                                             accs[h][1][:])
                        rec = smalls.tile([128, 1], F32, tag="rec", bufs=24,
                                          name="rec")
                        nc.vector.reciprocal(rec[:], ssum[:])
                        recs.append(rec)
                    return e_ts, recs

                def ensure(jt):
                    while emitted[0] <= jt:
                        cur = emitted[0]
                        if extra_work:
                            for w in extra_work.get(cur, ()):
                                w()
                        es[cur] = s_exp(cur)
                        emitted[0] += 1

                def s_exp_split2(hook):
                    # jts 0-1 phase-major by i-half, with `hook` (the b1
                    # qk-projection blocks) between the phases: the four
                    # ih0 exps give ACT ~4us of runway while the hook's
                    # matmuls wait on the second x half-column DMA.
                    ets = {jt: [epool.tile([128, N], F16, tag="e", name="e_t")
                                for _ in range(2)] for jt in (0, 1)}
                    accs = {(jt, h): [] for jt in (0, 1) for h in range(2)}
                    for ih in range(2):
                        for jt in (0, 1):
                            for h in range(2):
                                rp = 64 * h
                                s_ps = pp.tile([128, 1024], F32, tag="s",
                                               bufs=2, name="s_ps")
                                for q in range(2):
                                    ic = 2 * ih + q
                                    nc.tensor.matmul(
                                        s_ps[:, q * 512:(q + 1) * 512],
                                        lhsT=kk[rp:rp + 64,
                                                jt * 128:(jt + 1) * 128],
                                        rhs=qq[rp:rp + 64,
                                               ic * 512:(ic + 1) * 512],
                                        start=True, stop=True,
                                        tile_position=(rp, 0),
                                    )
                                acc = smalls.tile([128, 1], F32, tag="acc",
                                                  bufs=8, name="acc")
                                nc.scalar.activation(
                                    ets[jt][h][:, ih * 1024:(ih + 1) * 1024],
                                    s_ps[:], AF.Exp, scale=SCALE,
                                    accum_out=acc,
                                )
                                accs[(jt, h)].append(acc)
                        if ih == 0:
                            hook()
                    for jt in (0, 1):
                        recs = []
                        for h in range(2):
                            ssum = smalls.tile([128, 1], F32, tag="ssum",
                                               bufs=8, name="ssum")
                            nc.vector.tensor_add(ssum[:], accs[(jt, h)][0][:],
                                                 accs[(jt, h)][1][:])
                            rec = smalls.tile([128, 1], F32, tag="rec",
                                              bufs=24, name="rec")
                            nc.vector.reciprocal(rec[:], ssum[:])
                            recs.append(rec)
                        es[jt] = (ets[jt], recs)
                    emitted[0] = 2

                if jt0_hook is not None:
                    s_exp_split2(jt0_hook)
                else:
                    ensure(0)
                pre = list(prework)
                while pre:
                    for _ in range(2):
                        if pre:
                            pre.pop(0)()
                    if emitted[0] <= 14:
                        ensure(emitted[0])

                o_ps = pp.tile([128, N], F32, tag="o", bufs=1, name="o_ps")
                # zero-matmul: writes 0 everywhere and sets has_written on all
                # 4 banks, so every PV matmul can be a pure accumulate
                # (start=False) regardless of scheduling order.
                for ic in range(4):
                    nc.tensor.matmul(
                        o_ps[:, ic * 512:(ic + 1) * 512],
                        lhsT=zrow_sb[:1, :128], rhs=zrow_sb[:1, :],
                        start=True, stop=False, skip_group_check=True,
                    )

                for jt in range(16):
                    ensure(min(jt + 1, 15))  # S(jt+1) lands before PV(jt)
                    e_ts, recs = es.pop(jt)
                    vp = vppool.tile([128, 128], F16, tag="vp", name="vp")
                    for h in range(2):
                        nc.vector.tensor_scalar(
                            vp[:, h * 64:(h + 1) * 64],
                            v_sb[:, jt, (2 * p_ + h) * 64:(2 * p_ + h + 1) * 64],
                            recs[h][:], VP_SCALE, ALU.mult, ALU.mult,
                        )
                    for ic in range(4):
                        for h in range(2):
                            nc.tensor.matmul(
                                o_ps[64 * h:64 * (h + 1),
                                     ic * 512:(ic + 1) * 512],
                                lhsT=vp[:, h * 64:(h + 1) * 64],
                                rhs=e_ts[h][:, ic * 512:(ic + 1) * 512],
                                start=False,
                                stop=(jt == 15 and h == 1),
                                tile_position=(0, 64 * h),
                                skip_group_check=True,
                            )
                # evacuate O_T per 512-chunk (undo the VP_SCALE); pair 1's
                # tail evacuation goes partly on the (then idle) scalar engine
                for ic in range(4):
                    if p_ == 1 and ic % 2 == 0:
                        nc.scalar.mul(
                            o_sb[:, p_, ic * 512:(ic + 1) * 512],
                            o_ps[:, ic * 512:(ic + 1) * 512], 1.0 / VP_SCALE)
                    else:
                        nc.vector.tensor_scalar(
                            o_sb[:, p_, ic * 512:(ic + 1) * 512],
                            o_ps[:, ic * 512:(ic + 1) * 512],
                            1.0 / VP_SCALE, None, ALU.mult,
                        )
                    if p_ == 1:
                        for cot in range(4):
                            out_proj_unit(1, cot, ic, "s",
                                          "scalar" if cot % 2 == 0 else "vector")

            # ---- emission ----
            qk_proj(0, 0, "s")
            qk_proj(1, 0, "s", evac="scalar")
            v_proj(0, "s")

            def _b1_blocks():
                qk_proj(0, 1, "s")
                qk_proj(1, 1, "s")

            pre0 = [lambda f=ft, b=blk: qk_proj(f, b, "o")
                    for ft, blk in ((2, 0), (2, 1), (3, 0), (3, 1))]
            pre0 += [lambda n=nt: v_proj(n, "o") for nt in range(1, 16)]
            attention_pair(0, prework=pre0, jt0_hook=_b1_blocks)

            pre1 = [lambda c=cot, i=ic: out_proj_unit(0, c, i, "o", "vector")
                    for cot in range(4) for ic in range(4)]
            attention_pair(1, prework=pre1)

            # tail: pair 1's output units were emitted inside
            # attention_pair(1), each right after its evacuation chunk

    nc.compile()
    return nc


def get_nc():
    global _NC
    if _NC is None:
        _NC = _build_nc()
    return _NC


def core_inputs(x, Wp, bp, core):
    """Host-side shard prep for one core: b = core//2, g = core%2."""
    b, g = divmod(core, 2)
    # f' permutation: tiles [q0 q1 | k0 k1 | q2 q3 | k2 k3] (local heads)
    idx = []
    for pair in range(2):
        for which in (0, 1):  # q tile, then k tile
            for lh in (2 * pair, 2 * pair + 1):
                h = 4 * g + lh
                base = h * 192 + which * 64
                idx.extend(range(base, base + 64))
    idx = np.asarray(idx)
    vidx = []
    for lh in range(4):
        h = 4 * g + lh
        base = h * 192 + 128
        vidx.extend(range(base, base + 64))
    vidx = np.asarray(vidx)

    return {
        "x16": np.ascontiguousarray(x[b].astype(np.float16)),
        "wqk": np.ascontiguousarray(Wp[idx, :].T.astype(np.float16)),
        "bqk": np.ascontiguousarray(bp[idx].astype(np.float32).reshape(4, 128).T),
        "wv": np.ascontiguousarray(Wp[vidx, :].T.astype(np.float16)),
        "bpv": bp[vidx].astype(np.float16).reshape(1, 256),
        "ones": np.ones((1, 128), np.float16),
        "zrow": np.zeros((1, 512), np.float16),
    }


def kernel(x, Wp, bp, Wo, bo):
    global LAST_RESULT
    x = np.asarray(x, dtype=np.float32)
    Wp = np.asarray(Wp, dtype=np.float32)
    bp = np.asarray(bp, dtype=np.float32)
    Wo = np.asarray(Wo, dtype=np.float32)
    bo = np.asarray(bo, dtype=np.float32)

    in_maps = []
    for core in range(N_CORES):
        b, g = divmod(core, 2)
        m = core_inputs(x, Wp, bp, core)
        m["wo"] = np.ascontiguousarray(
            Wo[:, 256 * g:256 * (g + 1)].T.astype(np.float16))
        in_maps.append(m)

    nc = get_nc()
    res = run_bass_kernel_spmd(
        nc, in_maps, core_ids=list(range(N_CORES)),
        trace=bool(int(os.environ.get("KERNEL_TRACE", "0"))),
    )
    LAST_RESULT = res
    result = np.empty((B, C, N), dtype=np.float32)
    for b in range(B):
        r0, r1 = res.results[2 * b], res.results[2 * b + 1]
        result[b] = (
            r0["out_a"].astype(np.float32) + r0["out_b"].astype(np.float32)
            + r1["out_a"].astype(np.float32) + r1["out_b"].astype(np.float32)
            + x[b] + bo[:, None]
        )
    return result

